# revision 15
# baseline (speedup 1.0000x reference)
"""Trainium2 Bass kernel for nn_EnhancedLNN (feature-major data-parallel).

Contract: kernel(x, params) -> (logits [256,10] f32, probs [256,10] f32),
matching reference._forward's return tuple.

Strategy
--------
Pure data parallel over 8 NeuronCores (32 samples/core).  All activations are
kept FEATURE-MAJOR on device: a [D, B] tensor is stored as an SBUF tile
[128 partitions, (D/128 chunks) * 32 batch], so matmuls need no transposes
anywhere (out = W_aug.T-as-lhsT @ act-chunks).

Host-side preprocessing (free: not counted in HW time):
  * weights pre-transposed to in-major [K, M], cast to bf16
  * per-feature affine constants folded: BatchNorm (running stats) into the
    fusion linear; linear biases appended as an extra 128-row "aug" block
    whose first row pairs with a constant ones-row rhs chunk
  * x reshaped/transposed/chunked per core, cast to bf16

LayerNorm (gamma=1, beta=0 in this model -- asserted on host): feature
reductions are partition reductions in feature-major layout, computed with
ones-vector matmuls on the PE (sum and sum-of-squares accumulated across
chunks in PSUM), tiny DVE/ACT stat math, then a K=1 ones matmul broadcasts
mean/rstd across partitions for the normalize step.

ODE: fixed-step RK4 (dopri5 at rtol=1e-4 is ~1.1e-4 from truth; RK4 with
>=4 steps matches to the same floor).  Gate+ode weights concatenated to one
resident [640, 1024] bf16 matrix; one fused matmul pass per eval.
"""

import os
import sys
import numpy as np
import ml_dtypes

for _p in ("/opt/trn_rl_repo", "/root/.axon_site/_ro/trn_rl_repo"):
    if os.path.isdir(_p) and _p not in sys.path:
        sys.path.insert(0, _p)

import concourse.bacc as bacc
import concourse.bass as bass
import concourse.tile as tile
from concourse import mybir
from concourse.bass_utils import run_bass_kernel_spmd
from contextlib import ExitStack

# If BASS_TRACE is set but this container lacks the axon NTFF hook module,
# run_bass_kernel_spmd would crash importing it; provide a None-returning stub
# so it falls back to the untraced path instead.
try:
    import antenv.axon_hooks  # noqa: F401
except ImportError:
    import types
    import antenv
    _stub = types.ModuleType("antenv.axon_hooks")
    _stub.get_axon_ntff_profile_hook = lambda: None
    sys.modules["antenv.axon_hooks"] = _stub
    antenv.axon_hooks = _stub

BF16 = ml_dtypes.bfloat16
AF = mybir.ActivationFunctionType
ALU = mybir.AluOpType
DT = mybir.dt

NCORES = 8
B = 256
BL = B // NCORES            # 32 local batch
D_IN = 12288
LN_EPS = 1e-5
BN_EPS = 1e-5
NSTEPS = int(os.environ.get("BASS_ODE_STEPS", "2"))   # RK4 steps
T_END = 3.0

# layer table: name -> (n_in_chunks(without aug), n_out_chunks, has_aug)
# weight dram tensor "name" has shape [(nk_act + aug)*128, M]
LAYERS = {
    "w1":   (96, 16),
    "wb":   (16, 8),
    "wf":   (8, 16),
    "wv":   (16, 16),
    "w2":   (16, 8),
    "w3":   (8, 4),
    "wode": (4, 8),   # gate(512) | ode(512), aug only feeds gate bias
    "wc1":  (4, 2),
    "wc2":  (2, 0),   # M=10, special
}

LAST_RESULTS = None     # stash for test.py (exec_time_ns etc.)
_PROGRAM = None         # cached (nc,) build


def _f32(a):
    return np.asarray(a, np.float32)


def _prep_host(x, params):
    """Build device-layout arrays (shared across cores + per-core x)."""
    p = {k: _f32(v) for k, v in params.items()}

    # all LN affines in this model are identity; the kernel relies on it
    for g, be in (("g1", "be1"), ("g2", "be2"), ("g3", "be3"),
                  ("ng", "nbe"), ("gc", "bec")):
        assert np.all(p[g] == 1.0) and np.all(p[be] == 0.0), (g, be)
    assert np.all(p["bg"] == 1.0) and np.all(p["bbe"] == 0.0)

    dev = {}

    def waug(w_in_major, bias, name=None):
        """[K, M] in-major weights; bias as a separate [1, M] row tensor."""
        return (np.ascontiguousarray(np.asarray(w_in_major, np.float32)
                                     .astype(BF16)),
                np.ascontiguousarray(np.asarray(bias, np.float32)
                                     .astype(BF16).reshape(1, -1)))
    # L1: w1 [2048, 12288] out-major -> in-major [12288, 2048]
    dev["w1"], dev["w1_b"] = waug(p["w1"].T, p["b1"])
    # branches: 4x [256, 2048] -> concat out dim -> [2048, 1024]
    bw = np.concatenate([p["bw"][i].T for i in range(4)], axis=1)
    bb = np.concatenate([p["bb"][i] for i in range(4)])
    dev["wb"], dev["wb_b"] = waug(bw, bb)
    # fusion with BatchNorm folded (eval mode, running stats)
    s = p["bn_g"] / np.sqrt(p["bn_var"] + BN_EPS)
    fw = p["fw"].T * s[None, :]                      # [1024, 2048]
    fb = (p["fb"] - p["bn_mean"]) * s + p["bn_b"]
    dev["wf"], dev["wf_b"] = waug(fw, fb)
    # attention (seq len 1 => softmax == 1): two back-to-back linears with
    # no nonlinearity between them -- compose on host (exact, fp64):
    # attn = f @ (wo@wv).T + (wo@bv + bo)
    wv = p["attn_wqkv"][2 * 2048:3 * 2048].astype(np.float64)
    bv = p["attn_bqkv"][2 * 2048:3 * 2048].astype(np.float64)
    wo = p["attn_wo"].astype(np.float64)
    bo = p["attn_bo"].astype(np.float64)
    wc = (wo @ wv).astype(np.float32)
    bc = (wo @ bv + bo).astype(np.float32)
    dev["wv"], dev["wv_b"] = waug(wc.T, bc)
    dev["w2"], dev["w2_b"] = waug(p["w2"].T, p["b2"])
    dev["w3"], dev["w3_b"] = waug(p["w3"].T, p["b3"])
    # ODE: gate = sigmoid(y @ gw.T + gb); dy = gelu(y @ ode_w) * tc
    wcat = np.zeros((512, 1024), np.float32)
    wcat[:, :512] = p["gw"].T
    wcat[:, 512:] = p["ode_w"]
    dev["wode"] = np.ascontiguousarray(wcat.astype(BF16))
    ob = np.zeros((1, 1024), np.float32)
    ob[0, :512] = p["gb"]
    dev["wode_b"] = np.ascontiguousarray(ob.astype(BF16))
    dev["wc1"], dev["wc1_b"] = waug(p["wc1"].T, p["bc1"])
    dev["wc2"], dev["wc2_b"] = waug(p["wc2"].T, p["bc2"])

    # per-core x: [BL, 12288] -> feature-major chunks + aug ones chunk
    xf = _f32(x).reshape(B, D_IN)
    xs = []
    for r in range(NCORES):
        xr = xf[r * BL:(r + 1) * BL].T               # [12288, BL]
        xr = xr.reshape(96, 128, BL).transpose(1, 0, 2)   # [128, 96, BL]
        xs.append(np.ascontiguousarray(
            xr.reshape(128, 96 * BL).astype(BF16)))
    return dev, xs


# --------------------------------------------------------------------------
# device program
# --------------------------------------------------------------------------

class _Emit:
    def __init__(self, nc, tc, ctx):
        self.nc, self.tc, self.ctx = nc, tc, ctx
        P = tc.tile_pool
        self.wpools = {}
        self.act = ctx.enter_context(P(name="act", bufs=3))
        self.stat = ctx.enter_context(P(name="stat", bufs=10))
        self.keep = ctx.enter_context(P(name="keep", bufs=1))
        self.const = ctx.enter_context(P(name="const", bufs=1))
        self.pmain = ctx.enter_context(P(name="pmain", bufs=2, space="PSUM"))
        self.pm = ctx.enter_context(P(name="pm", bufs=2, space="PSUM"))
        self.pbc = ctx.enter_context(P(name="pbc", bufs=2, space="PSUM"))
        self.pode = ctx.enter_context(P(name="pode", bufs=2, space="PSUM"))

        nc_ = self.nc
        self.ones_bfrow = self.const.tile([1, BL], DT.bfloat16)
        nc_.gpsimd.memset(self.ones_bfrow[:], 1.0)
        self.ones_bf = self.const.tile([128, 1], DT.bfloat16)
        nc_.gpsimd.memset(self.ones_bf[:], 1.0)
        self.ones_row = self.const.tile([1, 128], DT.float32)
        nc_.gpsimd.memset(self.ones_row[:], 1.0)
        self.ones_col = self.const.tile([128, 1], DT.float32)
        nc_.gpsimd.memset(self.ones_col[:], 1.0)
        self.eps = self.const.tile([128, 1], DT.float32)
        nc_.gpsimd.memset(self.eps[:], LN_EPS)

    def wpool(self, key, bufs=3):
        if key not in self.wpools:
            self.wpools[key] = self.ctx.enter_context(
                self.tc.tile_pool(name=f"wp_{key}", bufs=bufs))
        return self.wpools[key]

    def linear(self, w_dram, rhs_tile, nk_act, nm, bias_dram=None,
               m_cols=None, psum_pool=None, dma_group=None,
               resident_tile=None, bias_tile=None):
        """Feature-major matmul: psum = W.T @ act (+ bias via K=1 matmul)."""
        nc = self.nc
        M = w_dram.shape[1]
        nk = nk_act
        if dma_group is None:
            dma_group = min(nk, max(1, (1 << 20) // (M * 2 * 128)))
        psum_pool = psum_pool or self.pmain
        out_parts = 128 if m_cols is None else M
        out_cols = m_cols if m_cols is not None else nm * BL
        psum = psum_pool.tile([out_parts, out_cols], DT.float32,
                              tag=psum_pool.name)
        wap = w_dram.ap().rearrange("(k p) m -> p k m", p=128)

        if bias_tile is None and bias_dram is not None:
            bias_tile = self.wpool("bias", bufs=4).tile(
                [1, M], DT.bfloat16, tag="bias")
            nc.sync.dma_start(bias_tile[:], bias_dram.ap())

        if resident_tile is not None:
            slabs = [(0, nk, resident_tile)]
        else:
            slabs = []
            pool = self.wpool(f"{M}_{dma_group}")
            for k0 in range(0, nk, dma_group):
                q = min(dma_group, nk - k0)
                t = pool.tile([128, dma_group, M], DT.bfloat16,
                              tag=f"w{M}_{dma_group}")
                nc.sync.dma_start(t[:, 0:q, :], wap[:, k0:k0 + q, :])
                slabs.append((k0, q, t))

        # bias first: its mj==0 matmul is the only start=True into this psum
        # tile.  HW: first_mm=1 clears has_written for the WHOLE bank; the
        # cleared bits make each region's first write an overwrite, later
        # ones accumulate -- so everything else uses start=False.
        nmj = nm if m_cols is None else 1
        if bias_tile is not None:
            for mj in range(nmj):
                if m_cols is None:
                    o = psum[:, BL * mj:BL * (mj + 1)]
                    bw = bias_tile[0:1, 128 * mj:128 * (mj + 1)]
                else:
                    o = psum[:, :]
                    bw = bias_tile[0:1, 0:M]
                nc.tensor.matmul(o, lhsT=bw, rhs=self.ones_bfrow[:],
                                 start=(mj == 0), stop=False,
                                 skip_group_check=True)
        for k0, q, t in slabs:
            for j in range(q):
                ki = k0 + j
                last = (ki == nk - 1)
                rhs = rhs_tile(ki) if callable(rhs_tile) \
                    else rhs_tile[:, BL * ki:BL * (ki + 1)]
                for mj in range(nmj):
                    if m_cols is None:
                        o = psum[:, BL * mj:BL * (mj + 1)]
                        w = t[:, j, 128 * mj:128 * (mj + 1)]
                    else:
                        o = psum[:, :]
                        w = t[:, j, 0:M]
                    nc.tensor.matmul(
                        o, lhsT=w, rhs=rhs,
                        start=(bias_tile is None and ki == 0 and mj == 0),
                        stop=last, skip_group_check=True)
        return psum

    def ln(self, tbf, nch, groups=1, out_dtype=DT.bfloat16, out_pool=None,
           out_tag=None):
        """LayerNorm over features (partitions x chunks) of tbf[128, nch*BL].

        groups: number of independent feature groups laid out contiguously
        (each nch//groups chunks).  Returns normalized tile [128, nch*BL].
        """
        nc = self.nc
        gch = nch // groups
        gw = groups * BL
        tsq = self.act.tile([128, nch * BL], DT.bfloat16, tag="tsq")
        nc.scalar.activation(tsq[:], tbf[:], AF.Square)
        mp = self.pm.tile([1, 2 * gw], DT.float32, tag="pm")
        for g in range(groups):
            for c in range(gch):
                ch = g * gch + c
                nc.tensor.matmul(mp[:, BL * g:BL * (g + 1)],
                                 lhsT=self.ones_bf[:],
                                 rhs=tbf[:, BL * ch:BL * (ch + 1)],
                                 start=(c == 0), stop=(c == gch - 1))
        for g in range(groups):
            for c in range(gch):
                ch = g * gch + c
                nc.tensor.matmul(mp[:, gw + BL * g:gw + BL * (g + 1)],
                                 lhsT=self.ones_bf[:],
                                 rhs=tsq[:, BL * ch:BL * (ch + 1)],
                                 start=(c == 0), stop=(c == gch - 1))
        inv_d = 1.0 / (gch * 128)
        mu = self.stat.tile([1, gw], DT.float32, tag="mu")
        ex2 = self.stat.tile([1, gw], DT.float32, tag="ex2")
        nc.vector.tensor_scalar(mu[:], mp[:, 0:gw], inv_d, None, ALU.mult)
        nc.vector.tensor_scalar(ex2[:], mp[:, gw:2 * gw], inv_d, None, ALU.mult)
        var = self.stat.tile([1, gw], DT.float32, tag="var")
        nc.vector.scalar_tensor_tensor(var[:], mu[:], -1.0, mu[:],
                                       ALU.mult, ALU.mult)      # -mu^2
        nc.vector.tensor_tensor(var[:], ex2[:], var[:], ALU.add)
        std = self.stat.tile([1, gw], DT.float32, tag="std")
        nc.scalar.activation(std[:], var[:], AF.Sqrt, bias=self.eps[0:1, :])
        rs = self.stat.tile([1, gw], DT.float32, tag="rs")
        nc.vector.reciprocal(rs[:], std[:])
        bc = self.pbc.tile([128, 2 * gw], DT.float32, tag="pbc")
        nc.tensor.matmul(bc[:, 0:gw], lhsT=self.ones_row[:], rhs=mu[:],
                         start=True, stop=True)
        nc.tensor.matmul(bc[:, gw:2 * gw], lhsT=self.ones_row[:], rhs=rs[:],
                         start=True, stop=True)
        pool = out_pool or self.act
        out = pool.tile([128, nch * BL], out_dtype, tag=out_tag or "norm")
        d = self.act.tile([128, nch * BL], DT.float32, tag="dtmp")
        for g in range(groups):
            sl = slice(g * gch * BL, (g + 1) * gch * BL)
            t3 = tbf[:, sl].rearrange("p (c b) -> p c b", c=gch)
            mu_b = bc[:, BL * g:BL * (g + 1)].unsqueeze(1) \
                .broadcast_to((128, gch, BL))
            rs_b = bc[:, gw + BL * g:gw + BL * (g + 1)].unsqueeze(1) \
                .broadcast_to((128, gch, BL))
            d3 = d[:, sl].rearrange("p (c b) -> p c b", c=gch)
            o3 = out[:, sl].rearrange("p (c b) -> p c b", c=gch)
            nc.vector.tensor_tensor(d3, t3, mu_b, ALU.subtract)
            nc.vector.tensor_tensor(o3, d3, rs_b, ALU.mult)
        return out


TAPS = [t for t in os.environ.get("BASS_TAPS", "").split(",") if t]


def _build_program():
    nc = bacc.Bacc("TRN2", target_bir_lowering=False, debug=False)
    tap_drams = {}

    def tap(name, tile_ap):
        if name not in TAPS:
            return
        d = nc.dram_tensor(f"tap_{name}", list(tile_ap.shape), tile_ap.dtype,
                           kind="ExternalOutput")
        tap_drams[name] = d
        nc.sync.dma_start(d.ap(), tile_ap)

    drams = {}
    for name, (nk, nm) in LAYERS.items():
        M = {"w1": 2048, "wb": 1024, "wf": 2048, "wv": 2048,
             "w2": 1024, "w3": 512, "wode": 1024, "wc1": 256, "wc2": 10}[name]
        drams[name] = nc.dram_tensor(name, [nk * 128, M], DT.bfloat16,
                                     kind="ExternalInput")
        drams[name + "_b"] = nc.dram_tensor(name + "_b", [1, M], DT.bfloat16,
                                            kind="ExternalInput")
    x_d = nc.dram_tensor("xdev", [128, 96 * BL], DT.bfloat16,
                         kind="ExternalInput")
    lg_d = nc.dram_tensor("logits_t", [10, BL], DT.float32,
                          kind="ExternalOutput")
    pr_d = nc.dram_tensor("probs_t", [10, BL], DT.float32,
                          kind="ExternalOutput")

    with tile.TileContext(nc) as tc, ExitStack() as ctx:
        em = _Emit(nc, tc, ctx)

        xt = em.keep.tile([128, 96 * BL], DT.bfloat16, tag="x")
        nc.sync.dma_start(xt[:], x_d.ap())

        # resident ODE weights
        wode_t = em.keep.tile([128, 4, 1024], DT.bfloat16, tag="wode")
        nc.sync.dma_start(
            wode_t[:], drams["wode"].ap().rearrange("(k p) m -> p k m", p=128))
        wode_b = em.keep.tile([1, 1024], DT.bfloat16, tag="wode_b")
        nc.sync.dma_start(wode_b[:], drams["wode_b"].ap())

        # ---- L1
        ps = em.linear(drams["w1"], xt, 96, 16, bias_dram=drams["w1_b"])
        t1 = em.act.tile([128, 16 * BL], DT.bfloat16, tag="t")
        nc.scalar.activation(t1[:], ps[:], AF.Gelu)
        tap("t1", t1[:])
        h1n = em.ln(t1, 16, out_pool=em.keep, out_tag="h1n")
        tap("h1n", h1n[:])

        # ---- branches (4x 2048->256, gelu, per-branch LN), concat
        ps = em.linear(drams["wb"], h1n, 16, 8, bias_dram=drams["wb_b"])
        tb = em.act.tile([128, 8 * BL], DT.bfloat16, tag="t")
        nc.scalar.activation(tb[:], ps[:], AF.Gelu)
        cn = em.ln(tb, 8, groups=4)
        tap("cn", cn[:])

        # ---- fusion + BN(folded) + gelu
        ps = em.linear(drams["wf"], cn, 8, 16, bias_dram=drams["wf_b"])
        f = em.act.tile([128, 16 * BL], DT.bfloat16, tag="t")
        nc.scalar.activation(f[:], ps[:], AF.Gelu)
        tap("f", f[:])

        # ---- attention (wo@wv composed on host)
        ps = em.linear(drams["wv"], f, 16, 16, bias_dram=drams["wv_b"])
        r = em.act.tile([128, 16 * BL], DT.bfloat16, tag="t")
        nc.vector.tensor_tensor(r[:], ps[:], h1n[:], ALU.add)
        rn = em.ln(r, 16)
        tap("rn", rn[:])

        # ---- w2, w3
        ps = em.linear(drams["w2"], rn, 16, 8, bias_dram=drams["w2_b"])
        t2 = em.act.tile([128, 8 * BL], DT.bfloat16, tag="t")
        nc.scalar.activation(t2[:], ps[:], AF.Gelu)
        h2n = em.ln(t2, 8)
        tap("h2n", h2n[:])
        ps = em.linear(drams["w3"], h2n, 8, 4, bias_dram=drams["w3_b"])
        t3 = em.act.tile([128, 4 * BL], DT.bfloat16, tag="t")
        nc.scalar.activation(t3[:], ps[:], AF.Gelu)
        y = em.ln(t3, 4, out_dtype=DT.float32, out_pool=em.keep, out_tag="y")
        ybf = em.keep.tile([128, 4 * BL], DT.bfloat16, tag="ybf")
        nc.vector.tensor_copy(ybf[:], y[:])
        tap("y0", y[:])

        # ---- ODE: RK4, dy = gelu(y@ode_w) * 1/(1+sig(sig(y@gw.T+gb)))
        dt_ = T_END / NSTEPS
        opool = ctx.enter_context(tc.tile_pool(name="ode", bufs=3))

        def ode_eval(ybf_t):
            pso = em.pode.tile([128, 8 * BL], DT.float32, tag="pode")
            # gate bias matmuls first (only start=True into this bank), then
            # gate weight m-chunks (0..3), then ode m-chunks (4..7, no bias)
            for mj in range(4):
                nc.tensor.matmul(pso[:, BL * mj:BL * (mj + 1)],
                                 lhsT=wode_b[0:1, 128 * mj:128 * (mj + 1)],
                                 rhs=em.ones_bfrow[:], start=(mj == 0),
                                 stop=False, skip_group_check=True)
            for mj in range(4):
                for ki in range(4):
                    nc.tensor.matmul(pso[:, BL * mj:BL * (mj + 1)],
                                     lhsT=wode_t[:, ki, 128 * mj:128 * (mj + 1)],
                                     rhs=ybf_t[:, BL * ki:BL * (ki + 1)],
                                     start=False, stop=(ki == 3),
                                     skip_group_check=True)
            for mj in range(4, 8):
                for ki in range(4):
                    nc.tensor.matmul(pso[:, BL * mj:BL * (mj + 1)],
                                     lhsT=wode_t[:, ki, 128 * mj:128 * (mj + 1)],
                                     rhs=ybf_t[:, BL * ki:BL * (ki + 1)],
                                     start=False, stop=(ki == 3),
                                     skip_group_check=True)
            # All three ACT calls stay in the sigmoid table set (sigmoid +
            # erf) -- a Gelu call would switch table sets (~2.7us each way).
            # gelu(z) = 0.5*z*(1+erf(z/sqrt2)); the 0.5 is folded into the
            # time constant: tc' = 0.5/(1+sig(sig(.))) via u = 2*s2+2.
            s1 = opool.tile([128, 4 * BL], DT.float32, tag="s1")
            nc.scalar.activation(s1[:], pso[:, 0:4 * BL], AF.Sigmoid)
            s2 = opool.tile([128, 4 * BL], DT.float32, tag="s2")
            nc.scalar.activation(s2[:], s1[:], AF.Sigmoid)
            u = opool.tile([128, 4 * BL], DT.float32, tag="u")
            nc.vector.tensor_scalar(u[:], s2[:], 2.0, 2.0, ALU.mult, ALU.add)
            tcn = opool.tile([128, 4 * BL], DT.float32, tag="tc")
            nc.vector.reciprocal(tcn[:], u[:])
            er = opool.tile([128, 4 * BL], DT.float32, tag="er")
            nc.scalar.activation(er[:], pso[:, 4 * BL:8 * BL], AF.Erf,
                                 scale=float(1.0 / np.sqrt(2.0)))
            a = opool.tile([128, 4 * BL], DT.float32, tag="a")
            nc.vector.scalar_tensor_tensor(a[:], er[:], 1.0,
                                           pso[:, 4 * BL:8 * BL],
                                           ALU.add, ALU.mult)
            k = opool.tile([128, 4 * BL], DT.float32, tag="k")
            nc.vector.tensor_tensor(k[:], a[:], tcn[:], ALU.mult)
            return k

        ycur, ycur_bf = y, ybf
        for _ in range(NSTEPS):
            k1 = ode_eval(ycur_bf)
            yt = opool.tile([128, 4 * BL], DT.bfloat16, tag="yt")
            nc.vector.scalar_tensor_tensor(yt[:], k1[:], dt_ / 2, ycur[:],
                                           ALU.mult, ALU.add)
            k2 = ode_eval(yt)
            acc = opool.tile([128, 4 * BL], DT.float32, tag="acc")
            nc.vector.scalar_tensor_tensor(acc[:], k2[:], 2.0, k1[:],
                                           ALU.mult, ALU.add)
            yt2 = opool.tile([128, 4 * BL], DT.bfloat16, tag="yt")
            nc.vector.scalar_tensor_tensor(yt2[:], k2[:], dt_ / 2, ycur[:],
                                           ALU.mult, ALU.add)
            k3 = ode_eval(yt2)
            acc2 = opool.tile([128, 4 * BL], DT.float32, tag="acc")
            nc.vector.scalar_tensor_tensor(acc2[:], k3[:], 2.0, acc[:],
                                           ALU.mult, ALU.add)
            yt3 = opool.tile([128, 4 * BL], DT.bfloat16, tag="yt")
            nc.vector.scalar_tensor_tensor(yt3[:], k3[:], dt_, ycur[:],
                                           ALU.mult, ALU.add)
            k4 = ode_eval(yt3)
            acc3 = opool.tile([128, 4 * BL], DT.float32, tag="acc")
            nc.vector.tensor_tensor(acc3[:], acc2[:], k4[:], ALU.add)
            ynew = em.keep.tile([128, 4 * BL], DT.float32, tag=f"y{_}")
            nc.vector.scalar_tensor_tensor(ynew[:], acc3[:], dt_ / 6, ycur[:],
                                           ALU.mult, ALU.add)
            ynew_bf = opool.tile([128, 4 * BL], DT.bfloat16, tag="ynbf")
            nc.vector.tensor_copy(ynew_bf[:], ynew[:])
            ycur, ycur_bf = ynew, ynew_bf

        tap("yend", ycur[:])

        # ---- classifier
        ps = em.linear(drams["wc1"], ycur_bf, 4, 2, bias_dram=drams["wc1_b"])
        erz = em.act.tile([128, 2 * BL], DT.float32, tag="erz")
        nc.scalar.activation(erz[:], ps[:], AF.Erf,
                             scale=float(1.0 / np.sqrt(2.0)))
        tz = em.act.tile([128, 2 * BL], DT.bfloat16, tag="t")
        nc.vector.scalar_tensor_tensor(tz[:], erz[:], 1.0, ps[:],
                                       ALU.add, ALU.mult)
        nc.vector.tensor_scalar(tz[:], tz[:], 0.5, None, ALU.mult)
        zn = em.ln(tz, 2)
        tap("zn", zn[:])
        # wc2: M=10
        psl = em.linear(drams["wc2"], zn, 2, 1, bias_dram=drams["wc2_b"],
                        m_cols=BL, psum_pool=em.pbc)
        lg = em.act.tile([10, BL], DT.float32, tag="lg")
        nc.scalar.activation(lg[:], psl[0:10, :], AF.Copy)
        nc.sync.dma_start(lg_d.ap(), lg[:])
        # softmax over 10 classes (partition dim): exp, ones-matmul sum,
        # reciprocal, broadcast, multiply.  |logits| <~ 3 so exp is safe.
        e = em.act.tile([10, BL], DT.float32, tag="e")
        nc.scalar.activation(e[:], psl[0:10, :], AF.Exp)
        se = em.pm.tile([1, BL], DT.float32, tag="pm")
        nc.tensor.matmul(se[:], lhsT=em.ones_col[0:10, :], rhs=e[:],
                         start=True, stop=True)
        ri = em.stat.tile([1, BL], DT.float32, tag="ri")
        nc.vector.reciprocal(ri[:], se[:])
        rb = em.pbc.tile([10, BL], DT.float32, tag="pbc")
        nc.tensor.matmul(rb[:], lhsT=em.ones_row[0:1, 0:10], rhs=ri[:],
                         start=True, stop=True)
        pr = em.act.tile([10, BL], DT.float32, tag="pr")
        nc.vector.tensor_tensor(pr[:], e[:], rb[:], ALU.mult)
        nc.sync.dma_start(pr_d.ap(), pr[:])

    nc.compile()
    return nc


def kernel(x, params):
    global _PROGRAM, LAST_RESULTS
    dev, xs = _prep_host(x, params)
    if _PROGRAM is None:
        _PROGRAM = _build_program()
    nc = _PROGRAM
    in_maps = []
    for r in range(NCORES):
        m = {k: np.asarray(v) for k, v in dev.items()}
        m["xdev"] = xs[r]
        in_maps.append(m)
    res = run_bass_kernel_spmd(nc, in_maps, core_ids=list(range(NCORES)))
    LAST_RESULTS = res
    logits = np.concatenate([res.results[r]["logits_t"].T
                             for r in range(NCORES)], axis=0)
    probs = np.concatenate([res.results[r]["probs_t"].T
                            for r in range(NCORES)], axis=0)
    return logits.astype(np.float32), probs.astype(np.float32)


# revision 18
# speedup vs baseline: 1.0133x; 1.0133x over previous
"""Trainium2 Bass kernel for nn_EnhancedLNN (feature-major data-parallel).

Contract: kernel(x, params) -> (logits [256,10] f32, probs [256,10] f32),
matching reference._forward's return tuple.

Strategy
--------
Pure data parallel over 8 NeuronCores (32 samples/core).  All activations are
kept FEATURE-MAJOR on device: a [D, B] tensor is stored as an SBUF tile
[128 partitions, (D/128 chunks) * 32 batch], so matmuls need no transposes
anywhere (out = W_aug.T-as-lhsT @ act-chunks).

Host-side preprocessing (free: not counted in HW time):
  * weights pre-transposed to in-major [K, M], cast to bf16
  * per-feature affine constants folded: BatchNorm (running stats) into the
    fusion linear; linear biases appended as an extra 128-row "aug" block
    whose first row pairs with a constant ones-row rhs chunk
  * x reshaped/transposed/chunked per core, cast to bf16

LayerNorm (gamma=1, beta=0 in this model -- asserted on host): feature
reductions are partition reductions in feature-major layout, computed with
ones-vector matmuls on the PE (sum and sum-of-squares accumulated across
chunks in PSUM), tiny DVE/ACT stat math, then a K=1 ones matmul broadcasts
mean/rstd across partitions for the normalize step.

ODE: fixed-step RK4 (dopri5 at rtol=1e-4 is ~1.1e-4 from truth; RK4 with
>=4 steps matches to the same floor).  Gate+ode weights concatenated to one
resident [640, 1024] bf16 matrix; one fused matmul pass per eval.
"""

import os
import sys
import numpy as np
import ml_dtypes

for _p in ("/opt/trn_rl_repo", "/root/.axon_site/_ro/trn_rl_repo"):
    if os.path.isdir(_p) and _p not in sys.path:
        sys.path.insert(0, _p)

import concourse.bacc as bacc
import concourse.bass as bass
import concourse.tile as tile
from concourse import mybir
from concourse.bass_utils import run_bass_kernel_spmd
from contextlib import ExitStack

# If BASS_TRACE is set but this container lacks the axon NTFF hook module,
# run_bass_kernel_spmd would crash importing it; provide a None-returning stub
# so it falls back to the untraced path instead.
try:
    import antenv.axon_hooks  # noqa: F401
except ImportError:
    import types
    import antenv
    _stub = types.ModuleType("antenv.axon_hooks")
    _stub.get_axon_ntff_profile_hook = lambda: None
    sys.modules["antenv.axon_hooks"] = _stub
    antenv.axon_hooks = _stub

BF16 = ml_dtypes.bfloat16
AF = mybir.ActivationFunctionType
ALU = mybir.AluOpType
DT = mybir.dt

NCORES = 8
B = 256
BL = B // NCORES            # 32 local batch
D_IN = 12288
LN_EPS = 1e-5
BN_EPS = 1e-5
NSTEPS = int(os.environ.get("BASS_ODE_STEPS", "2"))   # RK4 steps
T_END = 3.0

# layer table: name -> (n_in_chunks(without aug), n_out_chunks, has_aug)
# weight dram tensor "name" has shape [(nk_act + aug)*128, M]
LAYERS = {
    "w1":   (96, 16),
    "wb":   (16, 8),
    "wf":   (8, 16),
    "wv":   (16, 16),
    "w2":   (16, 8),
    "w3":   (8, 4),
    "wode": (4, 8),   # gate(512) | ode(512), aug only feeds gate bias
    "wc1":  (4, 2),
    "wc2":  (2, 0),   # M=10, special
}

LAST_RESULTS = None     # stash for test.py (exec_time_ns etc.)
_PROGRAM = None         # cached (nc,) build


def _f32(a):
    return np.asarray(a, np.float32)


def _prep_host(x, params):
    """Build device-layout arrays (shared across cores + per-core x)."""
    p = {k: _f32(v) for k, v in params.items()}

    # all LN affines in this model are identity; the kernel relies on it
    for g, be in (("g1", "be1"), ("g2", "be2"), ("g3", "be3"),
                  ("ng", "nbe"), ("gc", "bec")):
        assert np.all(p[g] == 1.0) and np.all(p[be] == 0.0), (g, be)
    assert np.all(p["bg"] == 1.0) and np.all(p["bbe"] == 0.0)

    dev = {}

    def waug(w_in_major, bias, name=None):
        """[K, M] in-major weights; bias as a separate [1, M] row tensor."""
        return (np.ascontiguousarray(np.asarray(w_in_major, np.float32)
                                     .astype(BF16)),
                np.ascontiguousarray(np.asarray(bias, np.float32)
                                     .astype(BF16).reshape(1, -1)))
    # L1: w1 [2048, 12288] out-major -> in-major [12288, 2048]
    dev["w1"], dev["w1_b"] = waug(p["w1"].T, p["b1"])
    # branches: 4x [256, 2048] -> concat out dim -> [2048, 1024]
    bw = np.concatenate([p["bw"][i].T for i in range(4)], axis=1)
    bb = np.concatenate([p["bb"][i] for i in range(4)])
    dev["wb"], dev["wb_b"] = waug(bw, bb)
    # fusion with BatchNorm folded (eval mode, running stats)
    s = p["bn_g"] / np.sqrt(p["bn_var"] + BN_EPS)
    fw = p["fw"].T * s[None, :]                      # [1024, 2048]
    fb = (p["fb"] - p["bn_mean"]) * s + p["bn_b"]
    dev["wf"], dev["wf_b"] = waug(fw, fb)
    # attention (seq len 1 => softmax == 1): two back-to-back linears with
    # no nonlinearity between them -- compose on host (exact, fp64):
    # attn = f @ (wo@wv).T + (wo@bv + bo)
    wv = p["attn_wqkv"][2 * 2048:3 * 2048].astype(np.float64)
    bv = p["attn_bqkv"][2 * 2048:3 * 2048].astype(np.float64)
    wo = p["attn_wo"].astype(np.float64)
    bo = p["attn_bo"].astype(np.float64)
    wc = (wo @ wv).astype(np.float32)
    bc = (wo @ bv + bo).astype(np.float32)
    dev["wv"], dev["wv_b"] = waug(wc.T, bc)
    dev["w2"], dev["w2_b"] = waug(p["w2"].T, p["b2"])
    dev["w3"], dev["w3_b"] = waug(p["w3"].T, p["b3"])
    # ODE: gate = sigmoid(y @ gw.T + gb); dy = gelu(y @ ode_w) * tc
    wcat = np.zeros((512, 1024), np.float32)
    wcat[:, :512] = p["gw"].T
    wcat[:, 512:] = p["ode_w"]
    dev["wode"] = np.ascontiguousarray(wcat.astype(BF16))
    ob = np.zeros((1, 1024), np.float32)
    ob[0, :512] = p["gb"]
    dev["wode_b"] = np.ascontiguousarray(ob.astype(BF16))
    dev["wc1"], dev["wc1_b"] = waug(p["wc1"].T, p["bc1"])
    dev["wc2"], dev["wc2_b"] = waug(p["wc2"].T, p["bc2"])

    # per-core x: [BL, 12288] -> feature-major chunks + aug ones chunk
    xf = _f32(x).reshape(B, D_IN)
    xs = []
    for r in range(NCORES):
        xr = xf[r * BL:(r + 1) * BL].T               # [12288, BL]
        xr = xr.reshape(96, 128, BL).transpose(1, 0, 2)   # [128, 96, BL]
        xs.append(np.ascontiguousarray(
            xr.reshape(128, 96 * BL).astype(BF16)))
    return dev, xs


# --------------------------------------------------------------------------
# device program
# --------------------------------------------------------------------------

class _Emit:
    def __init__(self, nc, tc, ctx):
        self.nc, self.tc, self.ctx = nc, tc, ctx
        P = tc.tile_pool
        self.wpools = {}
        self.act = ctx.enter_context(P(name="act", bufs=3))
        self.stat = ctx.enter_context(P(name="stat", bufs=10))
        self.keep = ctx.enter_context(P(name="keep", bufs=1))
        self.const = ctx.enter_context(P(name="const", bufs=1))
        self.pmain = ctx.enter_context(P(name="pmain", bufs=2, space="PSUM"))
        self.pm = ctx.enter_context(P(name="pm", bufs=1, space="PSUM"))
        self.pbc = ctx.enter_context(P(name="pbc", bufs=1, space="PSUM"))
        self.pode = ctx.enter_context(P(name="pode", bufs=4, space="PSUM"))

        nc_ = self.nc
        self.ones_bfrow = self.const.tile([1, BL], DT.bfloat16)
        nc_.gpsimd.memset(self.ones_bfrow[:], 1.0)
        self.ones_bf = self.const.tile([128, 1], DT.bfloat16)
        nc_.gpsimd.memset(self.ones_bf[:], 1.0)
        self.ones_row = self.const.tile([1, 128], DT.float32)
        nc_.gpsimd.memset(self.ones_row[:], 1.0)
        self.ones_col = self.const.tile([128, 1], DT.float32)
        nc_.gpsimd.memset(self.ones_col[:], 1.0)
        self.eps = self.const.tile([128, 1], DT.float32)
        nc_.gpsimd.memset(self.eps[:], LN_EPS)

    def wpool(self, key, bufs=3):
        if key not in self.wpools:
            self.wpools[key] = self.ctx.enter_context(
                self.tc.tile_pool(name=f"wp_{key}", bufs=bufs))
        return self.wpools[key]

    def linear(self, w_dram, rhs_tile, nk_act, nm, bias_dram=None,
               m_cols=None, psum_pool=None, dma_group=None,
               resident_tile=None, bias_tile=None):
        """Feature-major matmul: psum = W.T @ act (+ bias via K=1 matmul)."""
        nc = self.nc
        M = w_dram.shape[1]
        nk = nk_act
        if dma_group is None:
            dma_group = min(nk, max(1, (1 << 20) // (M * 2 * 128)))
        psum_pool = psum_pool or self.pmain
        out_parts = 128 if m_cols is None else M
        out_cols = m_cols if m_cols is not None else nm * BL
        psum = psum_pool.tile([out_parts, out_cols], DT.float32,
                              tag=psum_pool.name)
        wap = w_dram.ap().rearrange("(k p) m -> p k m", p=128)

        if bias_tile is None and bias_dram is not None:
            bias_tile = self.wpool("bias", bufs=4).tile(
                [1, M], DT.bfloat16, tag="bias")
            nc.sync.dma_start(bias_tile[:], bias_dram.ap())

        if resident_tile is not None:
            slabs = [(0, nk, resident_tile)]
        else:
            slabs = []
            pool = self.wpool(f"{M}_{dma_group}")
            for k0 in range(0, nk, dma_group):
                q = min(dma_group, nk - k0)
                t = pool.tile([128, dma_group, M], DT.bfloat16,
                              tag=f"w{M}_{dma_group}")
                nc.sync.dma_start(t[:, 0:q, :], wap[:, k0:k0 + q, :])
                slabs.append((k0, q, t))

        # bias first: its mj==0 matmul is the only start=True into this psum
        # tile.  HW: first_mm=1 clears has_written for the WHOLE bank; the
        # cleared bits make each region's first write an overwrite, later
        # ones accumulate -- so everything else uses start=False.
        nmj = nm if m_cols is None else 1
        if bias_tile is not None:
            for mj in range(nmj):
                if m_cols is None:
                    o = psum[:, BL * mj:BL * (mj + 1)]
                    bw = bias_tile[0:1, 128 * mj:128 * (mj + 1)]
                else:
                    o = psum[:, :]
                    bw = bias_tile[0:1, 0:M]
                nc.tensor.matmul(o, lhsT=bw, rhs=self.ones_bfrow[:],
                                 start=(mj == 0), stop=False,
                                 skip_group_check=True)
        for k0, q, t in slabs:
            for j in range(q):
                ki = k0 + j
                last = (ki == nk - 1)
                rhs = rhs_tile(ki) if callable(rhs_tile) \
                    else rhs_tile[:, BL * ki:BL * (ki + 1)]
                for mj in range(nmj):
                    if m_cols is None:
                        o = psum[:, BL * mj:BL * (mj + 1)]
                        w = t[:, j, 128 * mj:128 * (mj + 1)]
                    else:
                        o = psum[:, :]
                        w = t[:, j, 0:M]
                    nc.tensor.matmul(
                        o, lhsT=w, rhs=rhs,
                        start=(bias_tile is None and ki == 0 and mj == 0),
                        stop=last, skip_group_check=True)
        return psum

    def ln(self, tbf, nch, groups=1, out_dtype=DT.bfloat16, out_pool=None,
           out_tag=None):
        """LayerNorm over features (partitions x chunks) of tbf[128, nch*BL].

        groups: number of independent feature groups laid out contiguously
        (each nch//groups chunks).  Returns normalized tile [128, nch*BL].
        """
        nc = self.nc
        gch = nch // groups
        gw = groups * BL
        tsq = self.act.tile([128, nch * BL], DT.bfloat16, tag="tsq")
        nc.scalar.activation(tsq[:], tbf[:], AF.Square)
        mp = self.pm.tile([1, 2 * gw], DT.float32, tag="pm")
        for g in range(groups):
            for c in range(gch):
                ch = g * gch + c
                nc.tensor.matmul(mp[:, BL * g:BL * (g + 1)],
                                 lhsT=self.ones_bf[:],
                                 rhs=tbf[:, BL * ch:BL * (ch + 1)],
                                 start=(c == 0), stop=(c == gch - 1))
        for g in range(groups):
            for c in range(gch):
                ch = g * gch + c
                nc.tensor.matmul(mp[:, gw + BL * g:gw + BL * (g + 1)],
                                 lhsT=self.ones_bf[:],
                                 rhs=tsq[:, BL * ch:BL * (ch + 1)],
                                 start=(c == 0), stop=(c == gch - 1))
        inv_d = 1.0 / (gch * 128)
        mu = self.stat.tile([1, gw], DT.float32, tag="mu")
        ex2 = self.stat.tile([1, gw], DT.float32, tag="ex2")
        nc.vector.tensor_scalar(mu[:], mp[:, 0:gw], inv_d, None, ALU.mult)
        nc.vector.tensor_scalar(ex2[:], mp[:, gw:2 * gw], inv_d, None, ALU.mult)
        var = self.stat.tile([1, gw], DT.float32, tag="var")
        nc.vector.scalar_tensor_tensor(var[:], mu[:], -1.0, mu[:],
                                       ALU.mult, ALU.mult)      # -mu^2
        nc.vector.tensor_tensor(var[:], ex2[:], var[:], ALU.add)
        std = self.stat.tile([1, gw], DT.float32, tag="std")
        nc.scalar.activation(std[:], var[:], AF.Sqrt, bias=self.eps[0:1, :])
        rs = self.stat.tile([1, gw], DT.float32, tag="rs")
        nc.vector.reciprocal(rs[:], std[:])
        bc = self.pbc.tile([128, 2 * gw], DT.float32, tag="pbc")
        nc.tensor.matmul(bc[:, 0:gw], lhsT=self.ones_row[:], rhs=mu[:],
                         start=True, stop=True)
        nc.tensor.matmul(bc[:, gw:2 * gw], lhsT=self.ones_row[:], rhs=rs[:],
                         start=True, stop=True)
        pool = out_pool or self.act
        out = pool.tile([128, nch * BL], out_dtype, tag=out_tag or "norm")
        d = self.act.tile([128, nch * BL], DT.float32, tag="dtmp")
        for g in range(groups):
            sl = slice(g * gch * BL, (g + 1) * gch * BL)
            t3 = tbf[:, sl].rearrange("p (c b) -> p c b", c=gch)
            mu_b = bc[:, BL * g:BL * (g + 1)].unsqueeze(1) \
                .broadcast_to((128, gch, BL))
            rs_b = bc[:, gw + BL * g:gw + BL * (g + 1)].unsqueeze(1) \
                .broadcast_to((128, gch, BL))
            d3 = d[:, sl].rearrange("p (c b) -> p c b", c=gch)
            o3 = out[:, sl].rearrange("p (c b) -> p c b", c=gch)
            nc.vector.tensor_tensor(d3, t3, mu_b, ALU.subtract)
            nc.vector.tensor_tensor(o3, d3, rs_b, ALU.mult)
        return out


TAPS = [t for t in os.environ.get("BASS_TAPS", "").split(",") if t]


def _build_program():
    nc = bacc.Bacc("TRN2", target_bir_lowering=False, debug=False)
    tap_drams = {}

    def tap(name, tile_ap):
        if name not in TAPS:
            return
        d = nc.dram_tensor(f"tap_{name}", list(tile_ap.shape), tile_ap.dtype,
                           kind="ExternalOutput")
        tap_drams[name] = d
        nc.sync.dma_start(d.ap(), tile_ap)

    drams = {}
    for name, (nk, nm) in LAYERS.items():
        M = {"w1": 2048, "wb": 1024, "wf": 2048, "wv": 2048,
             "w2": 1024, "w3": 512, "wode": 1024, "wc1": 256, "wc2": 10}[name]
        drams[name] = nc.dram_tensor(name, [nk * 128, M], DT.bfloat16,
                                     kind="ExternalInput")
        drams[name + "_b"] = nc.dram_tensor(name + "_b", [1, M], DT.bfloat16,
                                            kind="ExternalInput")
    x_d = nc.dram_tensor("xdev", [128, 96 * BL], DT.bfloat16,
                         kind="ExternalInput")
    lg_d = nc.dram_tensor("logits_t", [10, BL], DT.float32,
                          kind="ExternalOutput")
    pr_d = nc.dram_tensor("probs_t", [10, BL], DT.float32,
                          kind="ExternalOutput")

    with tile.TileContext(nc) as tc, ExitStack() as ctx:
        em = _Emit(nc, tc, ctx)

        xt = em.keep.tile([128, 96 * BL], DT.bfloat16, tag="x")
        nc.sync.dma_start(xt[:], x_d.ap())

        # resident ODE weights
        wode_t = em.keep.tile([128, 4, 1024], DT.bfloat16, tag="wode")
        nc.sync.dma_start(
            wode_t[:], drams["wode"].ap().rearrange("(k p) m -> p k m", p=128))
        wode_b = em.keep.tile([1, 1024], DT.bfloat16, tag="wode_b")
        nc.sync.dma_start(wode_b[:], drams["wode_b"].ap())

        # ---- L1
        ps = em.linear(drams["w1"], xt, 96, 16, bias_dram=drams["w1_b"])
        t1 = em.act.tile([128, 16 * BL], DT.bfloat16, tag="t")
        nc.scalar.activation(t1[:], ps[:], AF.Gelu)
        tap("t1", t1[:])
        h1n = em.ln(t1, 16, out_pool=em.keep, out_tag="h1n")
        tap("h1n", h1n[:])

        # ---- branches (4x 2048->256, gelu, per-branch LN), concat
        ps = em.linear(drams["wb"], h1n, 16, 8, bias_dram=drams["wb_b"])
        tb = em.act.tile([128, 8 * BL], DT.bfloat16, tag="t")
        nc.scalar.activation(tb[:], ps[:], AF.Gelu)
        cn = em.ln(tb, 8, groups=4)
        tap("cn", cn[:])

        # ---- fusion + BN(folded) + gelu
        ps = em.linear(drams["wf"], cn, 8, 16, bias_dram=drams["wf_b"])
        f = em.act.tile([128, 16 * BL], DT.bfloat16, tag="t")
        nc.scalar.activation(f[:], ps[:], AF.Gelu)
        tap("f", f[:])

        # ---- attention (wo@wv composed on host)
        ps = em.linear(drams["wv"], f, 16, 16, bias_dram=drams["wv_b"])
        r = em.act.tile([128, 16 * BL], DT.bfloat16, tag="t")
        nc.vector.tensor_tensor(r[:], ps[:], h1n[:], ALU.add)
        rn = em.ln(r, 16)
        tap("rn", rn[:])

        # ---- w2, w3
        ps = em.linear(drams["w2"], rn, 16, 8, bias_dram=drams["w2_b"])
        t2 = em.act.tile([128, 8 * BL], DT.bfloat16, tag="t")
        nc.scalar.activation(t2[:], ps[:], AF.Gelu)
        h2n = em.ln(t2, 8)
        tap("h2n", h2n[:])
        ps = em.linear(drams["w3"], h2n, 8, 4, bias_dram=drams["w3_b"])
        t3 = em.act.tile([128, 4 * BL], DT.bfloat16, tag="t")
        nc.scalar.activation(t3[:], ps[:], AF.Gelu)
        y = em.ln(t3, 4, out_dtype=DT.float32, out_pool=em.keep, out_tag="y")
        ybf = em.keep.tile([128, 4 * BL], DT.bfloat16, tag="ybf")
        nc.vector.tensor_copy(ybf[:], y[:])
        tap("y0", y[:])

        # ---- ODE: RK4, dy = gelu(y@ode_w) * 1/(1+sig(sig(y@gw.T+gb)))
        # Two independent half-batch (16-sample) chains, interleaved so one
        # chain's serial ACT->DVE tail hides under the other's matmuls.
        dt_ = T_END / NSTEPS
        HB = BL // 2
        opool = ctx.enter_context(tc.tile_pool(name="ode", bufs=3))

        def ode_eval(yh, h):
            pso = em.pode.tile([128, 8 * HB], DT.float32, tag="pode")
            for mj in range(4):
                nc.tensor.matmul(pso[:, HB * mj:HB * (mj + 1)],
                                 lhsT=wode_b[0:1, 128 * mj:128 * (mj + 1)],
                                 rhs=em.ones_bfrow[0:1, 0:HB], start=(mj == 0),
                                 stop=False, skip_group_check=True)
            for mj in range(8):
                for ki in range(4):
                    nc.tensor.matmul(pso[:, HB * mj:HB * (mj + 1)],
                                     lhsT=wode_t[:, ki, 128 * mj:128 * (mj + 1)],
                                     rhs=yh[:, ki, :],
                                     start=False, stop=(ki == 3),
                                     skip_group_check=True)
            s1 = opool.tile([128, 4 * HB], DT.float32, tag=f"s1{h}")
            nc.scalar.activation(s1[:], pso[:, 0:4 * HB], AF.Sigmoid)
            s2 = opool.tile([128, 4 * HB], DT.float32, tag=f"s2{h}")
            nc.scalar.activation(s2[:], s1[:], AF.Sigmoid)
            u = opool.tile([128, 4 * HB], DT.float32, tag=f"u{h}")
            nc.vector.tensor_scalar(u[:], s2[:], 2.0, 2.0, ALU.mult, ALU.add)
            tcn = opool.tile([128, 4 * HB], DT.float32, tag=f"tc{h}")
            nc.vector.reciprocal(tcn[:], u[:])
            er = opool.tile([128, 4 * HB], DT.float32, tag=f"er{h}")
            nc.scalar.activation(er[:], pso[:, 4 * HB:8 * HB], AF.Erf,
                                 scale=float(1.0 / np.sqrt(2.0)))
            a = opool.tile([128, 4 * HB], DT.float32, tag=f"a{h}")
            nc.vector.scalar_tensor_tensor(a[:], er[:], 1.0,
                                           pso[:, 4 * HB:8 * HB],
                                           ALU.add, ALU.mult)
            k = opool.tile([128, 4 * HB], DT.float32, tag=f"k{h}")
            nc.vector.tensor_tensor(k[:].rearrange("p (c b) -> p c b", c=4),
                                    a[:].rearrange("p (c b) -> p c b", c=4),
                                    tcn[:].rearrange("p (c b) -> p c b", c=4),
                                    ALU.mult)
            return k

        def yview(t, h):
            # [128, 4, HB] strided view of half h of a full [128, 4*BL] tile
            return t[:].rearrange("p (c b) -> p c b", c=4)[:, :, HB * h:HB * (h + 1)]

        def kview(t):
            return t[:].rearrange("p (c b) -> p c b", c=4)

        ycur = y
        yh0 = [opool.tile([128, 4, HB], DT.bfloat16, tag=f"yh{h}",
                  name=f"yh0_{h}") for h in (0, 1)]
        for h in (0, 1):
            nc.vector.tensor_copy(yh0[h][:], yview(y, h))
        yhalves = yh0
        for _ in range(NSTEPS):
            ks = [[None, None] for _ in range(4)]
            accs = [None, None]
            stage_in = yhalves
            for st, coef in enumerate([dt_ / 2, dt_ / 2, dt_, None]):
                for h in (0, 1):
                    ks[st][h] = ode_eval(stage_in[h], h)
                if coef is not None:
                    nxt = [opool.tile([128, 4, HB], DT.bfloat16,
                                      tag=f"yt{h}", name=f"yt{st}_{h}")
                           for h in (0, 1)]
                    for h in (0, 1):
                        nc.vector.scalar_tensor_tensor(
                            nxt[h][:], kview(ks[st][h]), coef,
                            yview(ycur, h), ALU.mult, ALU.add)
                    stage_in = nxt
                if st == 1:
                    for h in (0, 1):
                        accs[h] = opool.tile([128, 4 * HB], DT.float32,
                                             tag=f"acc{h}", name=f"acc_{h}")
                        nc.vector.scalar_tensor_tensor(
                            accs[h][:], ks[1][h][:], 2.0, ks[0][h][:],
                            ALU.mult, ALU.add)
                elif st == 2:
                    for h in (0, 1):
                        nc.vector.scalar_tensor_tensor(
                            accs[h][:], ks[2][h][:], 2.0, accs[h][:],
                            ALU.mult, ALU.add)
            ynew = em.keep.tile([128, 4 * BL], DT.float32, tag=f"y{_}")
            nyh = [opool.tile([128, 4, HB], DT.bfloat16, tag=f"yh{h}",
                              name=f"nyh_{h}")
                   for h in (0, 1)]
            for h in (0, 1):
                nc.vector.tensor_tensor(accs[h][:], accs[h][:], ks[3][h][:],
                                        ALU.add)
                nc.vector.scalar_tensor_tensor(
                    yview(ynew, h), kview(accs[h]), dt_ / 6,
                    yview(ycur, h), ALU.mult, ALU.add)
                nc.vector.tensor_copy(nyh[h][:], yview(ynew, h))
            ycur, yhalves = ynew, nyh

        ycur_bf = em.keep.tile([128, 4 * BL], DT.bfloat16, tag="ybf_end")
        for h in (0, 1):
            nc.vector.tensor_copy(yview(ycur_bf, h), yhalves[h][:])
        tap("yend", ycur[:])

        # ---- classifier
        ps = em.linear(drams["wc1"], ycur_bf, 4, 2, bias_dram=drams["wc1_b"])
        erz = em.act.tile([128, 2 * BL], DT.float32, tag="erz")
        nc.scalar.activation(erz[:], ps[:], AF.Erf,
                             scale=float(1.0 / np.sqrt(2.0)))
        tz = em.act.tile([128, 2 * BL], DT.bfloat16, tag="t")
        nc.vector.scalar_tensor_tensor(tz[:], erz[:], 1.0, ps[:],
                                       ALU.add, ALU.mult)
        nc.vector.tensor_scalar(tz[:], tz[:], 0.5, None, ALU.mult)
        zn = em.ln(tz, 2)
        tap("zn", zn[:])
        # wc2: M=10
        psl = em.linear(drams["wc2"], zn, 2, 1, bias_dram=drams["wc2_b"],
                        m_cols=BL, psum_pool=em.pbc)
        lg = em.act.tile([10, BL], DT.float32, tag="lg")
        nc.scalar.activation(lg[:], psl[0:10, :], AF.Copy)
        nc.sync.dma_start(lg_d.ap(), lg[:])
        # softmax over 10 classes (partition dim): exp, ones-matmul sum,
        # reciprocal, broadcast, multiply.  |logits| <~ 3 so exp is safe.
        e = em.act.tile([10, BL], DT.float32, tag="e")
        nc.scalar.activation(e[:], psl[0:10, :], AF.Exp)
        se = em.pm.tile([1, BL], DT.float32, tag="pm")
        nc.tensor.matmul(se[:], lhsT=em.ones_col[0:10, :], rhs=e[:],
                         start=True, stop=True)
        ri = em.stat.tile([1, BL], DT.float32, tag="ri")
        nc.vector.reciprocal(ri[:], se[:])
        rb = em.pbc.tile([10, BL], DT.float32, tag="pbc")
        nc.tensor.matmul(rb[:], lhsT=em.ones_row[0:1, 0:10], rhs=ri[:],
                         start=True, stop=True)
        pr = em.act.tile([10, BL], DT.float32, tag="pr")
        nc.vector.tensor_tensor(pr[:], e[:], rb[:], ALU.mult)
        nc.sync.dma_start(pr_d.ap(), pr[:])

    nc.compile()
    return nc


def kernel(x, params):
    global _PROGRAM, LAST_RESULTS
    dev, xs = _prep_host(x, params)
    if _PROGRAM is None:
        _PROGRAM = _build_program()
    nc = _PROGRAM
    in_maps = []
    for r in range(NCORES):
        m = {k: np.asarray(v) for k, v in dev.items()}
        m["xdev"] = xs[r]
        in_maps.append(m)
    res = run_bass_kernel_spmd(nc, in_maps, core_ids=list(range(NCORES)))
    LAST_RESULTS = res
    logits = np.concatenate([res.results[r]["logits_t"].T
                             for r in range(NCORES)], axis=0)
    probs = np.concatenate([res.results[r]["probs_t"].T
                            for r in range(NCORES)], axis=0)
    return logits.astype(np.float32), probs.astype(np.float32)


# revision 19
# speedup vs baseline: 1.0187x; 1.0054x over previous
"""Trainium2 Bass kernel for nn_EnhancedLNN (feature-major data-parallel).

Contract: kernel(x, params) -> (logits [256,10] f32, probs [256,10] f32),
matching reference._forward's return tuple.

Strategy
--------
Pure data parallel over 8 NeuronCores (32 samples/core).  All activations are
kept FEATURE-MAJOR on device: a [D, B] tensor is stored as an SBUF tile
[128 partitions, (D/128 chunks) * 32 batch], so matmuls need no transposes
anywhere (out = W_aug.T-as-lhsT @ act-chunks).

Host-side preprocessing (free: not counted in HW time):
  * weights pre-transposed to in-major [K, M], cast to bf16
  * per-feature affine constants folded: BatchNorm (running stats) into the
    fusion linear; linear biases appended as an extra 128-row "aug" block
    whose first row pairs with a constant ones-row rhs chunk
  * x reshaped/transposed/chunked per core, cast to bf16

LayerNorm (gamma=1, beta=0 in this model -- asserted on host): feature
reductions are partition reductions in feature-major layout, computed with
ones-vector matmuls on the PE (sum and sum-of-squares accumulated across
chunks in PSUM), tiny DVE/ACT stat math, then a K=1 ones matmul broadcasts
mean/rstd across partitions for the normalize step.

ODE: fixed-step RK4 (dopri5 at rtol=1e-4 is ~1.1e-4 from truth; RK4 with
>=4 steps matches to the same floor).  Gate+ode weights concatenated to one
resident [640, 1024] bf16 matrix; one fused matmul pass per eval.
"""

import os
import sys
import numpy as np
import ml_dtypes

for _p in ("/opt/trn_rl_repo", "/root/.axon_site/_ro/trn_rl_repo"):
    if os.path.isdir(_p) and _p not in sys.path:
        sys.path.insert(0, _p)

import concourse.bacc as bacc
import concourse.bass as bass
import concourse.tile as tile
from concourse import mybir
from concourse.bass_utils import run_bass_kernel_spmd
from contextlib import ExitStack

# If BASS_TRACE is set but this container lacks the axon NTFF hook module,
# run_bass_kernel_spmd would crash importing it; provide a None-returning stub
# so it falls back to the untraced path instead.
try:
    import antenv.axon_hooks  # noqa: F401
except ImportError:
    import types
    import antenv
    _stub = types.ModuleType("antenv.axon_hooks")
    _stub.get_axon_ntff_profile_hook = lambda: None
    sys.modules["antenv.axon_hooks"] = _stub
    antenv.axon_hooks = _stub

BF16 = ml_dtypes.bfloat16
AF = mybir.ActivationFunctionType
ALU = mybir.AluOpType
DT = mybir.dt

NCORES = 8
B = 256
BL = B // NCORES            # 32 local batch
D_IN = 12288
LN_EPS = 1e-5
BN_EPS = 1e-5
NSTEPS = int(os.environ.get("BASS_ODE_STEPS", "2"))   # RK4 steps
T_END = 3.0

# layer table: name -> (n_in_chunks(without aug), n_out_chunks, has_aug)
# weight dram tensor "name" has shape [(nk_act + aug)*128, M]
LAYERS = {
    "w1":   (96, 16),
    "wb":   (16, 8),
    "wf":   (8, 16),
    "wv":   (16, 16),
    "w2":   (16, 8),
    "w3":   (8, 4),
    "wode": (4, 8),   # gate(512) | ode(512), aug only feeds gate bias
    "wc1":  (4, 2),
    "wc2":  (2, 0),   # M=10, special
}

LAST_RESULTS = None     # stash for test.py (exec_time_ns etc.)
_PROGRAM = None         # cached (nc,) build


def _f32(a):
    return np.asarray(a, np.float32)


def _prep_host(x, params):
    """Build device-layout arrays (shared across cores + per-core x)."""
    p = {k: _f32(v) for k, v in params.items()}

    # all LN affines in this model are identity; the kernel relies on it
    for g, be in (("g1", "be1"), ("g2", "be2"), ("g3", "be3"),
                  ("ng", "nbe"), ("gc", "bec")):
        assert np.all(p[g] == 1.0) and np.all(p[be] == 0.0), (g, be)
    assert np.all(p["bg"] == 1.0) and np.all(p["bbe"] == 0.0)

    dev = {}

    def waug(w_in_major, bias, name=None):
        """[K, M] in-major weights; bias as a separate [1, M] row tensor."""
        return (np.ascontiguousarray(np.asarray(w_in_major, np.float32)
                                     .astype(BF16)),
                np.ascontiguousarray(np.asarray(bias, np.float32)
                                     .astype(BF16).reshape(1, -1)))
    # L1: w1 [2048, 12288] out-major -> in-major [12288, 2048]
    dev["w1"], dev["w1_b"] = waug(p["w1"].T, p["b1"])
    # branches: 4x [256, 2048] -> concat out dim -> [2048, 1024]
    bw = np.concatenate([p["bw"][i].T for i in range(4)], axis=1)
    bb = np.concatenate([p["bb"][i] for i in range(4)])
    dev["wb"], dev["wb_b"] = waug(bw, bb)
    # fusion with BatchNorm folded (eval mode, running stats)
    s = p["bn_g"] / np.sqrt(p["bn_var"] + BN_EPS)
    fw = p["fw"].T * s[None, :]                      # [1024, 2048]
    fb = (p["fb"] - p["bn_mean"]) * s + p["bn_b"]
    dev["wf"], dev["wf_b"] = waug(fw, fb)
    # attention (seq len 1 => softmax == 1): two back-to-back linears with
    # no nonlinearity between them -- compose on host (exact, fp64):
    # attn = f @ (wo@wv).T + (wo@bv + bo)
    wv = p["attn_wqkv"][2 * 2048:3 * 2048].astype(np.float64)
    bv = p["attn_bqkv"][2 * 2048:3 * 2048].astype(np.float64)
    wo = p["attn_wo"].astype(np.float64)
    bo = p["attn_bo"].astype(np.float64)
    wc = (wo @ wv).astype(np.float32)
    bc = (wo @ bv + bo).astype(np.float32)
    dev["wv"], dev["wv_b"] = waug(wc.T, bc)
    dev["w2"], dev["w2_b"] = waug(p["w2"].T, p["b2"])
    dev["w3"], dev["w3_b"] = waug(p["w3"].T, p["b3"])
    # ODE: gate = sigmoid(y @ gw.T + gb); dy = gelu(y @ ode_w) * tc
    wcat = np.zeros((512, 1024), np.float32)
    wcat[:, :512] = p["gw"].T
    wcat[:, 512:] = p["ode_w"]
    dev["wode"] = np.ascontiguousarray(wcat.astype(BF16))
    ob = np.zeros((1, 1024), np.float32)
    ob[0, :512] = p["gb"]
    dev["wode_b"] = np.ascontiguousarray(ob.astype(BF16))
    dev["wc1"], dev["wc1_b"] = waug(p["wc1"].T, p["bc1"])
    dev["wc2"], dev["wc2_b"] = waug(p["wc2"].T, p["bc2"])

    # per-core x: [BL, 12288] -> feature-major chunks + aug ones chunk
    xf = _f32(x).reshape(B, D_IN)
    xs = []
    for r in range(NCORES):
        xr = xf[r * BL:(r + 1) * BL].T               # [12288, BL]
        xr = xr.reshape(96, 128, BL).transpose(1, 0, 2)   # [128, 96, BL]
        xs.append(np.ascontiguousarray(
            xr.reshape(128, 96 * BL).astype(BF16)))
    return dev, xs


# --------------------------------------------------------------------------
# device program
# --------------------------------------------------------------------------

class _Emit:
    def __init__(self, nc, tc, ctx):
        self.nc, self.tc, self.ctx = nc, tc, ctx
        P = tc.tile_pool
        self.wpools = {}
        self.act = ctx.enter_context(P(name="act", bufs=4))
        self.stat = ctx.enter_context(P(name="stat", bufs=10))
        self.keep = ctx.enter_context(P(name="keep", bufs=1))
        self.const = ctx.enter_context(P(name="const", bufs=1))
        self.pmain = ctx.enter_context(P(name="pmain", bufs=2, space="PSUM"))
        self.pm = ctx.enter_context(P(name="pm", bufs=1, space="PSUM"))
        self.pbc = ctx.enter_context(P(name="pbc", bufs=1, space="PSUM"))
        self.pode = ctx.enter_context(P(name="pode", bufs=4, space="PSUM"))

        nc_ = self.nc
        self.ones_bfrow = self.const.tile([1, BL], DT.bfloat16)
        nc_.gpsimd.memset(self.ones_bfrow[:], 1.0)
        self.ones_bf = self.const.tile([128, 1], DT.bfloat16)
        nc_.gpsimd.memset(self.ones_bf[:], 1.0)
        self.ones_row = self.const.tile([1, 128], DT.float32)
        nc_.gpsimd.memset(self.ones_row[:], 1.0)
        self.ones_col = self.const.tile([128, 1], DT.float32)
        nc_.gpsimd.memset(self.ones_col[:], 1.0)
        self.eps = self.const.tile([128, 1], DT.float32)
        nc_.gpsimd.memset(self.eps[:], LN_EPS)

    def wpool(self, key, bufs=3):
        if key not in self.wpools:
            self.wpools[key] = self.ctx.enter_context(
                self.tc.tile_pool(name=f"wp_{key}", bufs=bufs))
        return self.wpools[key]

    def linear(self, w_dram, rhs_tile, nk_act, nm, bias_dram=None,
               m_cols=None, psum_pool=None, dma_group=None,
               resident_tile=None, bias_tile=None):
        """Feature-major matmul: psum = W.T @ act (+ bias via K=1 matmul)."""
        nc = self.nc
        M = w_dram.shape[1]
        nk = nk_act
        if dma_group is None:
            dma_group = min(nk, max(1, (1 << 20) // (M * 2 * 128)))
        psum_pool = psum_pool or self.pmain
        out_parts = 128 if m_cols is None else M
        out_cols = m_cols if m_cols is not None else nm * BL
        psum = psum_pool.tile([out_parts, out_cols], DT.float32,
                              tag=psum_pool.name)
        wap = w_dram.ap().rearrange("(k p) m -> p k m", p=128)

        if bias_tile is None and bias_dram is not None:
            bias_tile = self.wpool("bias", bufs=4).tile(
                [1, M], DT.bfloat16, tag="bias")
            nc.sync.dma_start(bias_tile[:], bias_dram.ap())

        if resident_tile is not None:
            slabs = [(0, nk, resident_tile)]
        else:
            slabs = []
            pool = self.wpool(f"{M}_{dma_group}")
            for k0 in range(0, nk, dma_group):
                q = min(dma_group, nk - k0)
                t = pool.tile([128, dma_group, M], DT.bfloat16,
                              tag=f"w{M}_{dma_group}")
                nc.sync.dma_start(t[:, 0:q, :], wap[:, k0:k0 + q, :])
                slabs.append((k0, q, t))

        # bias first: its mj==0 matmul is the only start=True into this psum
        # tile.  HW: first_mm=1 clears has_written for the WHOLE bank; the
        # cleared bits make each region's first write an overwrite, later
        # ones accumulate -- so everything else uses start=False.
        nmj = nm if m_cols is None else 1
        if bias_tile is not None:
            for mj in range(nmj):
                if m_cols is None:
                    o = psum[:, BL * mj:BL * (mj + 1)]
                    bw = bias_tile[0:1, 128 * mj:128 * (mj + 1)]
                else:
                    o = psum[:, :]
                    bw = bias_tile[0:1, 0:M]
                nc.tensor.matmul(o, lhsT=bw, rhs=self.ones_bfrow[:],
                                 start=(mj == 0), stop=False,
                                 skip_group_check=True)
        for k0, q, t in slabs:
            for j in range(q):
                ki = k0 + j
                last = (ki == nk - 1)
                rhs = rhs_tile(ki) if callable(rhs_tile) \
                    else rhs_tile[:, BL * ki:BL * (ki + 1)]
                for mj in range(nmj):
                    if m_cols is None:
                        o = psum[:, BL * mj:BL * (mj + 1)]
                        w = t[:, j, 128 * mj:128 * (mj + 1)]
                    else:
                        o = psum[:, :]
                        w = t[:, j, 0:M]
                    nc.tensor.matmul(
                        o, lhsT=w, rhs=rhs,
                        start=(bias_tile is None and ki == 0 and mj == 0),
                        stop=last, skip_group_check=True)
        return psum

    def ln(self, tbf, nch, groups=1, out_dtype=DT.bfloat16, out_pool=None,
           out_tag=None):
        """LayerNorm over features (partitions x chunks) of tbf[128, nch*BL].

        groups: number of independent feature groups laid out contiguously
        (each nch//groups chunks).  Returns normalized tile [128, nch*BL].
        """
        nc = self.nc
        gch = nch // groups
        gw = groups * BL
        tsq = self.act.tile([128, nch * BL], DT.bfloat16, tag="tsq")
        nc.scalar.activation(tsq[:], tbf[:], AF.Square)
        mp = self.pm.tile([1, 2 * gw], DT.float32, tag="pm")
        for g in range(groups):
            for c in range(gch):
                ch = g * gch + c
                nc.tensor.matmul(mp[:, BL * g:BL * (g + 1)],
                                 lhsT=self.ones_bf[:],
                                 rhs=tbf[:, BL * ch:BL * (ch + 1)],
                                 start=(c == 0), stop=(c == gch - 1))
        for g in range(groups):
            for c in range(gch):
                ch = g * gch + c
                nc.tensor.matmul(mp[:, gw + BL * g:gw + BL * (g + 1)],
                                 lhsT=self.ones_bf[:],
                                 rhs=tsq[:, BL * ch:BL * (ch + 1)],
                                 start=(c == 0), stop=(c == gch - 1))
        inv_d = 1.0 / (gch * 128)
        mu = self.stat.tile([1, gw], DT.float32, tag="mu")
        ex2 = self.stat.tile([1, gw], DT.float32, tag="ex2")
        nc.vector.tensor_scalar(mu[:], mp[:, 0:gw], inv_d, None, ALU.mult)
        nc.vector.tensor_scalar(ex2[:], mp[:, gw:2 * gw], inv_d, None, ALU.mult)
        var = self.stat.tile([1, gw], DT.float32, tag="var")
        nc.vector.scalar_tensor_tensor(var[:], mu[:], -1.0, mu[:],
                                       ALU.mult, ALU.mult)      # -mu^2
        nc.vector.tensor_tensor(var[:], ex2[:], var[:], ALU.add)
        std = self.stat.tile([1, gw], DT.float32, tag="std")
        nc.scalar.activation(std[:], var[:], AF.Sqrt, bias=self.eps[0:1, :])
        rs = self.stat.tile([1, gw], DT.float32, tag="rs")
        nc.vector.reciprocal(rs[:], std[:])
        bc = self.pbc.tile([128, 2 * gw], DT.float32, tag="pbc")
        nc.tensor.matmul(bc[:, 0:gw], lhsT=self.ones_row[:], rhs=mu[:],
                         start=True, stop=True)
        nc.tensor.matmul(bc[:, gw:2 * gw], lhsT=self.ones_row[:], rhs=rs[:],
                         start=True, stop=True)
        pool = out_pool or self.act
        out = pool.tile([128, nch * BL], out_dtype, tag=out_tag or "norm")
        d = self.act.tile([128, nch * BL], DT.float32, tag="dtmp")
        for g in range(groups):
            sl = slice(g * gch * BL, (g + 1) * gch * BL)
            t3 = tbf[:, sl].rearrange("p (c b) -> p c b", c=gch)
            mu_b = bc[:, BL * g:BL * (g + 1)].unsqueeze(1) \
                .broadcast_to((128, gch, BL))
            rs_b = bc[:, gw + BL * g:gw + BL * (g + 1)].unsqueeze(1) \
                .broadcast_to((128, gch, BL))
            d3 = d[:, sl].rearrange("p (c b) -> p c b", c=gch)
            o3 = out[:, sl].rearrange("p (c b) -> p c b", c=gch)
            nc.vector.tensor_tensor(d3, t3, mu_b, ALU.subtract)
            nc.vector.tensor_tensor(o3, d3, rs_b, ALU.mult)
        return out


TAPS = [t for t in os.environ.get("BASS_TAPS", "").split(",") if t]


def _build_program():
    nc = bacc.Bacc("TRN2", target_bir_lowering=False, debug=False)
    tap_drams = {}

    def tap(name, tile_ap):
        if name not in TAPS:
            return
        d = nc.dram_tensor(f"tap_{name}", list(tile_ap.shape), tile_ap.dtype,
                           kind="ExternalOutput")
        tap_drams[name] = d
        nc.sync.dma_start(d.ap(), tile_ap)

    drams = {}
    for name, (nk, nm) in LAYERS.items():
        M = {"w1": 2048, "wb": 1024, "wf": 2048, "wv": 2048,
             "w2": 1024, "w3": 512, "wode": 1024, "wc1": 256, "wc2": 10}[name]
        drams[name] = nc.dram_tensor(name, [nk * 128, M], DT.bfloat16,
                                     kind="ExternalInput")
        drams[name + "_b"] = nc.dram_tensor(name + "_b", [1, M], DT.bfloat16,
                                            kind="ExternalInput")
    x_d = nc.dram_tensor("xdev", [128, 96 * BL], DT.bfloat16,
                         kind="ExternalInput")
    lg_d = nc.dram_tensor("logits_t", [10, BL], DT.float32,
                          kind="ExternalOutput")
    pr_d = nc.dram_tensor("probs_t", [10, BL], DT.float32,
                          kind="ExternalOutput")

    with tile.TileContext(nc) as tc, ExitStack() as ctx:
        em = _Emit(nc, tc, ctx)

        xt = em.keep.tile([128, 96 * BL], DT.bfloat16, tag="x")
        nc.sync.dma_start(xt[:], x_d.ap())

        # resident ODE weights
        wode_t = em.keep.tile([128, 4, 1024], DT.bfloat16, tag="wode")
        nc.sync.dma_start(
            wode_t[:], drams["wode"].ap().rearrange("(k p) m -> p k m", p=128))
        wode_b = em.keep.tile([1, 1024], DT.bfloat16, tag="wode_b")
        nc.sync.dma_start(wode_b[:], drams["wode_b"].ap())

        # ---- L1
        ps = em.linear(drams["w1"], xt, 96, 16, bias_dram=drams["w1_b"])
        t1 = em.act.tile([128, 16 * BL], DT.bfloat16, tag="t")
        nc.scalar.activation(t1[:], ps[:], AF.Gelu)
        tap("t1", t1[:])
        h1n = em.ln(t1, 16, out_pool=em.keep, out_tag="h1n")
        tap("h1n", h1n[:])

        # ---- branches (4x 2048->256, gelu, per-branch LN), concat
        ps = em.linear(drams["wb"], h1n, 16, 8, bias_dram=drams["wb_b"])
        tb = em.act.tile([128, 8 * BL], DT.bfloat16, tag="t")
        nc.scalar.activation(tb[:], ps[:], AF.Gelu)
        cn = em.ln(tb, 8, groups=4)
        tap("cn", cn[:])

        # ---- fusion + BN(folded) + gelu
        ps = em.linear(drams["wf"], cn, 8, 16, bias_dram=drams["wf_b"])
        f = em.act.tile([128, 16 * BL], DT.bfloat16, tag="t")
        nc.scalar.activation(f[:], ps[:], AF.Gelu)
        tap("f", f[:])

        # ---- attention (wo@wv composed on host)
        ps = em.linear(drams["wv"], f, 16, 16, bias_dram=drams["wv_b"])
        r = em.act.tile([128, 16 * BL], DT.bfloat16, tag="t")
        nc.vector.tensor_tensor(r[:], ps[:], h1n[:], ALU.add)
        rn = em.ln(r, 16)
        tap("rn", rn[:])

        # ---- w2, w3
        ps = em.linear(drams["w2"], rn, 16, 8, bias_dram=drams["w2_b"])
        t2 = em.act.tile([128, 8 * BL], DT.bfloat16, tag="t")
        nc.scalar.activation(t2[:], ps[:], AF.Gelu)
        h2n = em.ln(t2, 8)
        tap("h2n", h2n[:])
        ps = em.linear(drams["w3"], h2n, 8, 4, bias_dram=drams["w3_b"])
        t3 = em.act.tile([128, 4 * BL], DT.bfloat16, tag="t")
        nc.scalar.activation(t3[:], ps[:], AF.Gelu)
        y = em.ln(t3, 4, out_dtype=DT.float32, out_pool=em.keep, out_tag="y")
        ybf = em.keep.tile([128, 4 * BL], DT.bfloat16, tag="ybf")
        nc.vector.tensor_copy(ybf[:], y[:])
        tap("y0", y[:])

        # ---- ODE: RK4, dy = gelu(y@ode_w) * 1/(1+sig(sig(y@gw.T+gb)))
        # Two independent half-batch (16-sample) chains, interleaved so one
        # chain's serial ACT->DVE tail hides under the other's matmuls.
        dt_ = T_END / NSTEPS
        HB = BL // 2
        opool = ctx.enter_context(tc.tile_pool(name="ode", bufs=4))

        def ode_eval(yh, h):
            pso = em.pode.tile([128, 8 * HB], DT.float32, tag="pode")
            for mj in range(4):
                nc.tensor.matmul(pso[:, HB * mj:HB * (mj + 1)],
                                 lhsT=wode_b[0:1, 128 * mj:128 * (mj + 1)],
                                 rhs=em.ones_bfrow[0:1, 0:HB], start=(mj == 0),
                                 stop=False, skip_group_check=True)
            for mj in range(8):
                for ki in range(4):
                    nc.tensor.matmul(pso[:, HB * mj:HB * (mj + 1)],
                                     lhsT=wode_t[:, ki, 128 * mj:128 * (mj + 1)],
                                     rhs=yh[:, ki, :],
                                     start=False, stop=(ki == 3),
                                     skip_group_check=True)
            s1 = opool.tile([128, 4 * HB], DT.float32, tag=f"s1{h}")
            nc.scalar.activation(s1[:], pso[:, 0:4 * HB], AF.Sigmoid)
            s2 = opool.tile([128, 4 * HB], DT.float32, tag=f"s2{h}")
            nc.scalar.activation(s2[:], s1[:], AF.Sigmoid)
            u = opool.tile([128, 4 * HB], DT.float32, tag=f"u{h}")
            nc.vector.tensor_scalar(u[:], s2[:], 2.0, 2.0, ALU.mult, ALU.add)
            tcn = opool.tile([128, 4 * HB], DT.float32, tag=f"tc{h}")
            nc.vector.reciprocal(tcn[:], u[:])
            er = opool.tile([128, 4 * HB], DT.float32, tag=f"er{h}")
            nc.scalar.activation(er[:], pso[:, 4 * HB:8 * HB], AF.Erf,
                                 scale=float(1.0 / np.sqrt(2.0)))
            a = opool.tile([128, 4 * HB], DT.float32, tag=f"a{h}")
            nc.vector.scalar_tensor_tensor(a[:], er[:], 1.0,
                                           pso[:, 4 * HB:8 * HB],
                                           ALU.add, ALU.mult)
            k = opool.tile([128, 4 * HB], DT.float32, tag=f"k{h}")
            nc.vector.tensor_tensor(k[:].rearrange("p (c b) -> p c b", c=4),
                                    a[:].rearrange("p (c b) -> p c b", c=4),
                                    tcn[:].rearrange("p (c b) -> p c b", c=4),
                                    ALU.mult)
            return k

        def yview(t, h):
            # [128, 4, HB] strided view of half h of a full [128, 4*BL] tile
            return t[:].rearrange("p (c b) -> p c b", c=4)[:, :, HB * h:HB * (h + 1)]

        def kview(t):
            return t[:].rearrange("p (c b) -> p c b", c=4)

        ycur = y
        yh0 = [opool.tile([128, 4, HB], DT.bfloat16, tag=f"yh{h}",
                  name=f"yh0_{h}") for h in (0, 1)]
        for h in (0, 1):
            nc.vector.tensor_copy(yh0[h][:], yview(y, h))
        yhalves = yh0
        for _ in range(NSTEPS):
            ks = [[None, None] for _ in range(4)]
            accs = [None, None]
            stage_in = yhalves
            for st, coef in enumerate([dt_ / 2, dt_ / 2, dt_, None]):
                for h in (0, 1):
                    ks[st][h] = ode_eval(stage_in[h], h)
                if coef is not None:
                    nxt = [opool.tile([128, 4, HB], DT.bfloat16,
                                      tag=f"yt{h}", name=f"yt{st}_{h}")
                           for h in (0, 1)]
                    for h in (0, 1):
                        nc.vector.scalar_tensor_tensor(
                            nxt[h][:], kview(ks[st][h]), coef,
                            yview(ycur, h), ALU.mult, ALU.add)
                    stage_in = nxt
                if st == 1:
                    for h in (0, 1):
                        accs[h] = opool.tile([128, 4 * HB], DT.float32,
                                             tag=f"acc{h}", name=f"acc_{h}")
                        nc.vector.scalar_tensor_tensor(
                            accs[h][:], ks[1][h][:], 2.0, ks[0][h][:],
                            ALU.mult, ALU.add)
                elif st == 2:
                    for h in (0, 1):
                        nc.vector.scalar_tensor_tensor(
                            accs[h][:], ks[2][h][:], 2.0, accs[h][:],
                            ALU.mult, ALU.add)
            ynew = em.keep.tile([128, 4 * BL], DT.float32, tag=f"y{_}")
            nyh = [opool.tile([128, 4, HB], DT.bfloat16, tag=f"yh{h}",
                              name=f"nyh_{h}")
                   for h in (0, 1)]
            for h in (0, 1):
                nc.vector.tensor_tensor(accs[h][:], accs[h][:], ks[3][h][:],
                                        ALU.add)
                nc.vector.scalar_tensor_tensor(
                    yview(ynew, h), kview(accs[h]), dt_ / 6,
                    yview(ycur, h), ALU.mult, ALU.add)
                nc.vector.tensor_copy(nyh[h][:], yview(ynew, h))
            ycur, yhalves = ynew, nyh

        ycur_bf = em.keep.tile([128, 4 * BL], DT.bfloat16, tag="ybf_end")
        for h in (0, 1):
            nc.vector.tensor_copy(yview(ycur_bf, h), yhalves[h][:])
        tap("yend", ycur[:])

        # ---- classifier
        ps = em.linear(drams["wc1"], ycur_bf, 4, 2, bias_dram=drams["wc1_b"])
        erz = em.act.tile([128, 2 * BL], DT.float32, tag="erz")
        nc.scalar.activation(erz[:], ps[:], AF.Erf,
                             scale=float(1.0 / np.sqrt(2.0)))
        tz = em.act.tile([128, 2 * BL], DT.bfloat16, tag="t")
        nc.vector.scalar_tensor_tensor(tz[:], erz[:], 1.0, ps[:],
                                       ALU.add, ALU.mult)
        nc.vector.tensor_scalar(tz[:], tz[:], 0.5, None, ALU.mult)
        zn = em.ln(tz, 2)
        tap("zn", zn[:])
        # wc2: M=10
        psl = em.linear(drams["wc2"], zn, 2, 1, bias_dram=drams["wc2_b"],
                        m_cols=BL, psum_pool=em.pbc)
        lg = em.act.tile([10, BL], DT.float32, tag="lg")
        nc.scalar.activation(lg[:], psl[0:10, :], AF.Copy)
        nc.sync.dma_start(lg_d.ap(), lg[:])
        # softmax over 10 classes (partition dim): exp, ones-matmul sum,
        # reciprocal, broadcast, multiply.  |logits| <~ 3 so exp is safe.
        e = em.act.tile([10, BL], DT.float32, tag="e")
        nc.scalar.activation(e[:], psl[0:10, :], AF.Exp)
        se = em.pm.tile([1, BL], DT.float32, tag="pm")
        nc.tensor.matmul(se[:], lhsT=em.ones_col[0:10, :], rhs=e[:],
                         start=True, stop=True)
        ri = em.stat.tile([1, BL], DT.float32, tag="ri")
        nc.vector.reciprocal(ri[:], se[:])
        rb = em.pbc.tile([10, BL], DT.float32, tag="pbc")
        nc.tensor.matmul(rb[:], lhsT=em.ones_row[0:1, 0:10], rhs=ri[:],
                         start=True, stop=True)
        pr = em.act.tile([10, BL], DT.float32, tag="pr")
        nc.vector.tensor_tensor(pr[:], e[:], rb[:], ALU.mult)
        nc.sync.dma_start(pr_d.ap(), pr[:])

    nc.compile()
    return nc


def kernel(x, params):
    global _PROGRAM, LAST_RESULTS
    dev, xs = _prep_host(x, params)
    if _PROGRAM is None:
        _PROGRAM = _build_program()
    nc = _PROGRAM
    in_maps = []
    for r in range(NCORES):
        m = {k: np.asarray(v) for k, v in dev.items()}
        m["xdev"] = xs[r]
        in_maps.append(m)
    res = run_bass_kernel_spmd(nc, in_maps, core_ids=list(range(NCORES)))
    LAST_RESULTS = res
    logits = np.concatenate([res.results[r]["logits_t"].T
                             for r in range(NCORES)], axis=0)
    probs = np.concatenate([res.results[r]["probs_t"].T
                            for r in range(NCORES)], axis=0)
    return logits.astype(np.float32), probs.astype(np.float32)
